# revision 31
# baseline (speedup 1.0000x reference)
"""Distributed dot-product attention (with attention-weights output) for one
TRN2 chip (8 NeuronCores), Bass/Tile.

Problem:  scores = keys @ queries.T           [Nk=4096, Nq=4096]  (no scale)
          attn   = softmax(scores, axis=0)    (over keys)
          weighted = attn.T @ values          [Nq=4096, d=1024]
          returns (weighted, attn)

Sharding: queries split along Nq across the 8 cores (512 queries each);
keys/values replicated. Each core computes its 512-column block of attn
([4096, 512]) and of weighted ([512, 1024]); results are concatenated.

Per-core kernel (all fp32 data, matmuls in fp32r = full-rate PE mode):
  phase 0: PE-transpose Q block -> QT [d, q]
  phase A: per 512-key panel: PE-transpose K panel -> KT [d, k]; mm1
           scores_T tile [q=128, k=512] = QT.T @ KT accumulated over d;
           track per-panel row max; stash scores in SBUF.
  softmax: exp(s - max) via ACT with accumulated row sums, then scale by
           1/sum (exact softmax; fp32r only affects the matmul inputs).
  phase A3: PE-transpose attn [q,k] -> attnT [k,q] tiles (kept in SBUF,
           rounded to fp32r) and DMA them out as the attn output.
  phase B: weighted.T block: psum[q,e] += attnT_tile.T @ V_tile over all
           32 key tiles (fp32r), then copy/DMA out.
"""
import sys

if "/opt/trn_rl_repo" not in sys.path:
    sys.path.insert(0, "/opt/trn_rl_repo")

import numpy as np

# ---------------------------------------------------------------------------
# Patch: this container's walrus build only accepts ONE sync-wait command per
# instruction; Tile attaches several. Split multi-wait instructions into
# single-wait nops (same engine, immediately before) + the original with one.
# ---------------------------------------------------------------------------
import bass_rust
import concourse.tile as tile_mod
from concourse import mybir

_ScopedClock = bass_rust.ScopedClock
_SyncInfo = bass_rust.SyncInfo
_split_counter = [0]


def _split_waits_in_ordered(ordered):
    for bb_name, insts in ordered.items():
        if not any(
            i.sync_info is not None and len(i.sync_info.on_wait) > 1 for i in insts
        ):
            continue
        new_list = []
        for inst in insts:
            si = inst.sync_info
            if si is not None and len(si.on_wait) > 1:
                waits = list(si.on_wait)
                for w in waits[:-1]:
                    _split_counter[0] += 1
                    nop = mybir.InstNoOp(
                        name=f"SW-{_split_counter[0]}-{inst.name}", ins=[], outs=[]
                    )
                    nop.engine = inst.engine
                    nop.sync_info = _SyncInfo(on_wait=[w], on_update=[])
                    nop.bass_scheduled_tick = inst.bass_scheduled_tick
                    nop.bass_scheduled_proc = inst.bass_scheduled_proc
                    nop.bass_scheduled_scope = inst.bass_scheduled_scope
                    new_list.append(nop)
                inst.sync_info = _SyncInfo(
                    on_wait=[waits[-1]], on_update=list(si.on_update)
                )
            new_list.append(inst)
        insts[:] = new_list


def _apply_tile_patch():
    TileContext = tile_mod.TileContext
    if getattr(TileContext, "_drain_patch_applied", False):
        return

    orig_lower = TileContext._lower_ordered_insts

    def _lower_ordered_insts(self, ordered):
        _split_waits_in_ordered(ordered)
        return orig_lower(self, ordered)

    TileContext._lower_ordered_insts = _lower_ordered_insts

    def _drain_and_barrier(self, tick_clock, wait_clock):
        nc = self.nc
        probe = nc.sync.nop(nofuse=True)
        wait_clock.add_sem_waits(
            probe.ins, _ScopedClock({None: tick_clock.global_clock})
        )
        si = probe.ins.sync_info
        if si is not None and len(si.on_wait) > 1:
            waits = list(si.on_wait)
            probe.ins.sync_info = _SyncInfo(
                on_wait=[waits[0]], on_update=list(si.on_update)
            )
            for w in waits[1:]:
                nop = nc.sync.nop(nofuse=True)
                nop.ins.sync_info = _SyncInfo(on_wait=[w], on_update=[])
        nc.sync.drain()

        nc.all_engine_barrier()
        assert self.sems is not None
        popped = nc._tile_sem_poison_stack.pop()
        assert popped is self._sem_poison
        nc.clear_and_free_semaphores(list(self.sems.allocated().values()))
        nc.all_engine_barrier()

    TileContext._drain_and_barrier = _drain_and_barrier
    TileContext._drain_patch_applied = True


_apply_tile_patch()

import concourse.bass as bass
import concourse.tile as tile
from concourse import masks
from concourse.bass_utils import run_bass_kernel_spmd

F32 = mybir.dt.float32
F32R = mybir.dt.float32r

# problem shape (hardcoded per spec)
NQ, NK, D = 4096, 4096, 1024
N_CORES = 8
NQL = NQ // N_CORES          # 512 queries per core
P = 128                      # partitions
PANEL = 512                  # keys per mm1 panel
NQT = NQL // P               # 4 q tiles per core
NPAN = NK // PANEL           # 8 key panels
NDC = D // P                 # 8 d chunks
NKT = NK // P                # 32 key tiles
EH = 512                     # e half width
NEH = D // EH                # 2


def build_attention_nc():
    nc = bass.Bass()
    qT = nc.declare_dram_parameter("qT", [D, NQL], F32, isOutput=False)
    kT = nc.declare_dram_parameter("kT", [D, NK], F32, isOutput=False)
    values = nc.declare_dram_parameter("values", [NK, D], F32, isOutput=False)
    attn = nc.declare_dram_parameter("attn", [NK, NQL], F32, isOutput=True)
    weighted = nc.declare_dram_parameter("weighted", [NQL, D], F32, isOutput=True)

    with tile.TileContext(nc) as tc:
        from contextlib import ExitStack

        with ExitStack() as octx:
            cpool = octx.enter_context(tc.tile_pool(name="const", bufs=1))
            ident = cpool.tile([P, P], F32)
            masks.make_identity(nc, ident[:])

            atp = octx.enter_context(tc.tile_pool(name="attnT", bufs=1))
            attnT = atp.tile([P, NKT, NQL], F32R)  # [k-part, k-tile, q]

            stats = octx.enter_context(tc.tile_pool(name="stats", bufs=1))
            mp = stats.tile([P, NQT, NPAN], F32)      # per-panel row maxes
            negmp = stats.tile([P, NQT, NPAN], F32)   # -mp (exp bias)
            psums = stats.tile([P, NQT, NPAN], F32)   # per-panel exp row sums
            mfin = stats.tile([P, NQT], F32)
            dlt = stats.tile([P, NQT, NPAN], F32)     # mp - mfin
            epan = stats.tile([P, NQT, NPAN], F32)    # exp(mp - mfin)
            wpan = stats.tile([P, NQT, NPAN], F32)    # epan * psums
            ssum = stats.tile([P, NQT], F32)
            recip = stats.tile([P, NQT], F32)
            scl = stats.tile([P, NQT, NPAN], F32)     # epan / ssum

            kldp = octx.enter_context(tc.tile_pool(name="kld", bufs=1))
            vrp = octx.enter_context(tc.tile_pool(name="vr", bufs=3))

            psx = ExitStack()
            scp = psx.enter_context(tc.tile_pool(name="scores", bufs=1))
            psmm = psx.enter_context(tc.tile_pool(name="psmm", bufs=2, space="PSUM"))
            pstp = psx.enter_context(tc.tile_pool(name="pstp", bufs=4, space="PSUM"))

            with ExitStack() as actx:
                qtp = actx.enter_context(tc.tile_pool(name="qt", bufs=1))
                ktp = actx.enter_context(tc.tile_pool(name="kt", bufs=2))

                # ---- phase 0: load Q^T (pre-transposed on host), round ----
                QT = qtp.tile([P, NDC, NQL], F32R)   # [d-part, d-chunk, q]

                qT_r = qT.rearrange("(c p) q -> p c q", p=P)
                kT_r = kT.rearrange("(c p) k -> p c k", p=P)

                def load_qt_pair(dcp):
                    # two d-chunks per DMA (4KB/partition)
                    qst = kldp.tile([P, 2, NQL], F32, tag=f"kld{dcp % 4}", name="qst")
                    nc.sync.dma_start(qst[:], qT_r[:, dcp:dcp + 2, :])
                    if dcp % 4 == 0:
                        nc.vector.tensor_copy(QT[:, dcp:dcp + 2, :], qst[:])
                    else:
                        nc.scalar.copy(QT[:, dcp:dcp + 2, :], qst[:])

                # ---- phase A: per-panel K^T load+round, then mm1 ----
                scores = [
                    scp.tile([P, NK], F32, tag=f"sc{qi}", name=f"sc{qi}") for qi in range(NQT)
                ]

                def load_round_pair(kt_tile, pa, dcp):
                    # two d-chunks per DMA (2x2KB strided per partition)
                    kst = kldp.tile([P, 2, PANEL], F32, tag=f"kld{dcp % 4}", name="kst")
                    nc.sync.dma_start(
                        kst[:],
                        kT_r[:, dcp:dcp + 2, pa * PANEL:(pa + 1) * PANEL],
                    )
                    if dcp % 4 == 0:
                        nc.vector.tensor_copy(kt_tile[:, dcp:dcp + 2, :], kst[:])
                    else:
                        nc.scalar.copy(kt_tile[:, dcp:dcp + 2, :], kst[:])

                def load_round_panel(kt_tile, pa):
                    for dcp in range(0, NDC, 2):
                        load_round_pair(kt_tile, pa, dcp)

                def mm1_quarter(kt_tile, pa, qi, half):
                    # 4 accumulating matmuls (half a d-sweep) for one q tile
                    s = mm_psum[qi]
                    for dc in range(half * 4, half * 4 + 4):
                        nc.tensor.matmul(
                            s[:],
                            QT[:, dc, qi * P:(qi + 1) * P],
                            kt_tile[:, dc, :],
                            start=(dc == 0),
                            stop=(dc == NDC - 1),
                        )
                    if half == 1:
                        nc.vector.reduce_max(
                            mp[:, qi, pa:pa + 1], s[:], axis=mybir.AxisListType.X
                        )
                        nc.vector.tensor_scalar_mul(
                            negmp[:, qi, pa:pa + 1], mp[:, qi, pa:pa + 1], -1.0
                        )
                        nc.scalar.activation(
                            scores[qi][:, pa * PANEL:(pa + 1) * PANEL],
                            s[:],
                            mybir.ActivationFunctionType.Exp,
                            bias=negmp[:, qi, pa:pa + 1],
                            scale=1.0,
                            accum_out=psums[:, qi, pa:pa + 1],
                        )

                mm_psum = {}
                KT_tiles = {}
                KT_tiles[0] = ktp.tile([P, NDC, PANEL], F32R, tag="kt", name="kt0")
                # first-needed chunks first: K panel-0 dc 0-3 and Q dc 0-3,
                # so the first mm1 quarter can start after ~4 fat transfers
                for dcp in (0, 2):
                    load_round_pair(KT_tiles[0], 0, dcp)
                for dcp in (0, 2):
                    load_qt_pair(dcp)
                for dcp in (4, 6):
                    load_round_pair(KT_tiles[0], 0, dcp)
                for dcp in (4, 6):
                    load_qt_pair(dcp)
                for pa in range(NPAN):
                    if pa + 1 < NPAN:
                        KT_tiles[pa + 1] = ktp.tile(
                            [P, NDC, PANEL], F32R, tag="kt", name=f"kt{pa+1}"
                        )
                        load_round_panel(KT_tiles[pa + 1], pa + 1)
                    # fresh psum accumulators per panel
                    for qi in range(NQT):
                        mm_psum[qi] = psmm.tile(
                            [P, PANEL], F32, tag=f"mm1_{qi % 2}", name=f"s{qi}"
                        )
                    for g in range(8):
                        qi, half = divmod(g, 2)
                        mm1_quarter(KT_tiles[pa], pa, qi, half)
                    del KT_tiles[pa]

                # ---- softmax + attn transpose, per q tile (overlapped) ----
                for qi in range(NQT):
                    nc.vector.reduce_max(
                        mfin[:, qi:qi + 1], mp[:, qi, :], axis=mybir.AxisListType.X
                    )
                    # scale_p = exp(mp - mfin) * psums normalization factors
                    nc.vector.tensor_scalar(
                        dlt[:, qi, :], mp[:, qi, :], mfin[:, qi:qi + 1], None,
                        op0=mybir.AluOpType.subtract,
                    )
                    nc.scalar.activation(
                        epan[:, qi, :], dlt[:, qi, :],
                        mybir.ActivationFunctionType.Exp,
                    )
                    nc.vector.tensor_tensor(
                        wpan[:, qi, :], epan[:, qi, :], psums[:, qi, :],
                        op=mybir.AluOpType.mult,
                    )
                    nc.vector.reduce_sum(
                        ssum[:, qi:qi + 1], wpan[:, qi, :], axis=mybir.AxisListType.X
                    )
                    nc.vector.reciprocal(recip[:, qi:qi + 1], ssum[:, qi:qi + 1])
                    nc.vector.tensor_scalar_mul(
                        scl[:, qi, :], epan[:, qi, :], recip[:, qi:qi + 1]
                    )
                    # rescale + transpose per 512-col chunk (chunk == panel)
                    for group in range(NKT // 4):
                        sl = slice(group * 4 * P, (group + 1) * 4 * P)
                        eng = (
                            nc.vector.tensor_scalar_mul
                            if group % 2 == 0
                            else nc.scalar.mul
                        )
                        eng(
                            scores[qi][:, sl], scores[qi][:, sl],
                            scl[:, qi, group:group + 1],
                        )
                        pt = pstp.tile([P, 4 * P], F32, tag="tp", name="pt")
                        for j in range(4):
                            kt_i = group * 4 + j
                            nc.tensor.transpose(
                                pt[:, j * P:(j + 1) * P],
                                scores[qi][:, kt_i * P:(kt_i + 1) * P],
                                ident[:],
                            )
                        eng = nc.vector.tensor_copy if group % 2 == 0 else nc.scalar.copy
                        eng(
                            attnT[:, group * 4:(group + 1) * 4, qi * P:(qi + 1) * P],
                            pt[:].rearrange("p (j q) -> p j q", j=4),
                        )
                        if qi == NQT - 1:
                            # tile rows complete once the last q block lands
                            for j in range(4):
                                kt_i = group * 4 + j
                                nc.gpsimd.dma_start(
                                    attn[kt_i * P:(kt_i + 1) * P, :],
                                    attnT[:, kt_i, :].bitcast(F32),
                                )
            psx.close()

            # ---- phase B: weighted = attnT.T @ V ----
            with ExitStack() as bctx:
                woutp = bctx.enter_context(tc.tile_pool(name="wout", bufs=4))
                psb = bctx.enter_context(
                    tc.tile_pool(name="psb", bufs=1, space="PSUM")
                )
                acc = [
                    [psb.tile([P, EH], F32, tag=f"acc{eh}_{qi}", name=f"acc{eh}_{qi}") for qi in range(NQT)]
                    for eh in range(NEH)
                ]
                for kt_i in range(NKT):
                    vld = kldp.tile([P, D], F32, tag=f"kld{kt_i % 4}", name="vld")
                    nc.sync.dma_start(vld[:], values[kt_i * P:(kt_i + 1) * P, :])
                    vr = vrp.tile([P, D], F32R, tag="vr")
                    if kt_i % 2 == 0:
                        nc.vector.tensor_copy(vr[:], vld[:])
                    else:
                        nc.scalar.copy(vr[:], vld[:])
                    for eh in range(NEH):
                        for qi in range(NQT):
                            nc.tensor.matmul(
                                acc[eh][qi][:],
                                attnT[:, kt_i, qi * P:(qi + 1) * P],
                                vr[:, eh * EH:(eh + 1) * EH],
                                start=(kt_i == 0),
                                stop=(kt_i == NKT - 1),
                            )
                for eh in range(NEH):
                    for qi in range(NQT):
                        wout = woutp.tile([P, EH], F32, tag="wout")
                        nc.vector.tensor_copy(wout[:], acc[eh][qi][:])
                        nc.scalar.dma_start(
                            weighted[qi * P:(qi + 1) * P, eh * EH:(eh + 1) * EH],
                            wout[:],
                        )
    return nc


_NC_CACHE = {}


def _get_nc():
    if "nc" not in _NC_CACHE:
        _NC_CACHE["nc"] = build_attention_nc()
    return _NC_CACHE["nc"]


def make_in_maps(queries, keys, values):
    keys_T = np.ascontiguousarray(keys.T)
    queries_T = np.ascontiguousarray(queries.T)
    return [
        {
            "qT": np.ascontiguousarray(queries_T[:, c * NQL:(c + 1) * NQL]),
            "kT": keys_T,
            "values": values,
        }
        for c in range(N_CORES)
    ]


def kernel(queries, keys, values):
    queries = np.ascontiguousarray(np.asarray(queries, dtype=np.float32))
    keys = np.ascontiguousarray(np.asarray(keys, dtype=np.float32))
    values = np.ascontiguousarray(np.asarray(values, dtype=np.float32))
    assert queries.shape == (NQ, D) and keys.shape == (NK, D)
    assert values.shape == (NK, D)

    nc = _get_nc()
    in_maps = make_in_maps(queries, keys, values)
    res = run_bass_kernel_spmd(nc, in_maps, list(range(N_CORES)))
    attn_full = np.concatenate(
        [res.results[c]["attn"] for c in range(N_CORES)], axis=1
    )
    weighted_full = np.concatenate(
        [res.results[c]["weighted"] for c in range(N_CORES)], axis=0
    )
    return (weighted_full, attn_full)


# revision 32
# speedup vs baseline: 1.1749x; 1.1749x over previous
"""Distributed dot-product attention (with attention-weights output) for one
TRN2 chip (8 NeuronCores), Bass/Tile.

Problem:  scores = keys @ queries.T           [Nk=4096, Nq=4096]  (no scale)
          attn   = softmax(scores, axis=0)    (over keys)
          weighted = attn.T @ values          [Nq=4096, d=1024]
          returns (weighted, attn)

Sharding: queries split along Nq across the 8 cores (512 queries each);
keys/values replicated. Each core computes its 512-column block of attn
([4096, 512]) and of weighted ([512, 1024]); results are concatenated.

Per-core kernel (all fp32 data, matmuls in fp32r = full-rate PE mode):
  phase 0: PE-transpose Q block -> QT [d, q]
  phase A: per 512-key panel: PE-transpose K panel -> KT [d, k]; mm1
           scores_T tile [q=128, k=512] = QT.T @ KT accumulated over d;
           track per-panel row max; stash scores in SBUF.
  softmax: exp(s - max) via ACT with accumulated row sums, then scale by
           1/sum (exact softmax; fp32r only affects the matmul inputs).
  phase A3: PE-transpose attn [q,k] -> attnT [k,q] tiles (kept in SBUF,
           rounded to fp32r) and DMA them out as the attn output.
  phase B: weighted.T block: psum[q,e] += attnT_tile.T @ V_tile over all
           32 key tiles (fp32r), then copy/DMA out.
"""
import sys

if "/opt/trn_rl_repo" not in sys.path:
    sys.path.insert(0, "/opt/trn_rl_repo")

import numpy as np

# ---------------------------------------------------------------------------
# Patch: this container's walrus build only accepts ONE sync-wait command per
# instruction; Tile attaches several. Split multi-wait instructions into
# single-wait nops (same engine, immediately before) + the original with one.
# ---------------------------------------------------------------------------
import bass_rust
import concourse.tile as tile_mod
from concourse import mybir

_ScopedClock = bass_rust.ScopedClock
_SyncInfo = bass_rust.SyncInfo
_split_counter = [0]


def _split_waits_in_ordered(ordered):
    for bb_name, insts in ordered.items():
        if not any(
            i.sync_info is not None and len(i.sync_info.on_wait) > 1 for i in insts
        ):
            continue
        new_list = []
        for inst in insts:
            si = inst.sync_info
            if si is not None and len(si.on_wait) > 1:
                waits = list(si.on_wait)
                for w in waits[:-1]:
                    _split_counter[0] += 1
                    nop = mybir.InstNoOp(
                        name=f"SW-{_split_counter[0]}-{inst.name}", ins=[], outs=[]
                    )
                    nop.engine = inst.engine
                    nop.sync_info = _SyncInfo(on_wait=[w], on_update=[])
                    nop.bass_scheduled_tick = inst.bass_scheduled_tick
                    nop.bass_scheduled_proc = inst.bass_scheduled_proc
                    nop.bass_scheduled_scope = inst.bass_scheduled_scope
                    new_list.append(nop)
                inst.sync_info = _SyncInfo(
                    on_wait=[waits[-1]], on_update=list(si.on_update)
                )
            new_list.append(inst)
        insts[:] = new_list


def _apply_tile_patch():
    TileContext = tile_mod.TileContext
    if getattr(TileContext, "_drain_patch_applied", False):
        return

    orig_lower = TileContext._lower_ordered_insts

    def _lower_ordered_insts(self, ordered):
        _split_waits_in_ordered(ordered)
        return orig_lower(self, ordered)

    TileContext._lower_ordered_insts = _lower_ordered_insts

    def _drain_and_barrier(self, tick_clock, wait_clock):
        nc = self.nc
        probe = nc.sync.nop(nofuse=True)
        wait_clock.add_sem_waits(
            probe.ins, _ScopedClock({None: tick_clock.global_clock})
        )
        si = probe.ins.sync_info
        if si is not None and len(si.on_wait) > 1:
            waits = list(si.on_wait)
            probe.ins.sync_info = _SyncInfo(
                on_wait=[waits[0]], on_update=list(si.on_update)
            )
            for w in waits[1:]:
                nop = nc.sync.nop(nofuse=True)
                nop.ins.sync_info = _SyncInfo(on_wait=[w], on_update=[])
        nc.sync.drain()

        nc.all_engine_barrier()
        assert self.sems is not None
        popped = nc._tile_sem_poison_stack.pop()
        assert popped is self._sem_poison
        nc.clear_and_free_semaphores(list(self.sems.allocated().values()))
        nc.all_engine_barrier()

    TileContext._drain_and_barrier = _drain_and_barrier
    TileContext._drain_patch_applied = True


_apply_tile_patch()

import concourse.bass as bass
import concourse.tile as tile
from concourse import masks
from concourse.bass_utils import run_bass_kernel_spmd

F32 = mybir.dt.float32
F32R = mybir.dt.float32r

# problem shape (hardcoded per spec)
NQ, NK, D = 4096, 4096, 1024
N_CORES = 8
NQL = NQ // N_CORES          # 512 queries per core
P = 128                      # partitions
PANEL = 512                  # keys per mm1 panel
NQT = NQL // P               # 4 q tiles per core
NPAN = NK // PANEL           # 8 key panels
NDC = D // P                 # 8 d chunks
NKT = NK // P                # 32 key tiles
EH = 512                     # e half width
NEH = D // EH                # 2


def build_attention_nc():
    nc = bass.Bass()
    qT = nc.declare_dram_parameter("qT", [D, NQL], F32, isOutput=False)
    kT = nc.declare_dram_parameter("kT", [D, NK], F32, isOutput=False)
    values = nc.declare_dram_parameter("values", [NK, D], F32, isOutput=False)
    attn = nc.declare_dram_parameter("attn", [NK, NQL], F32, isOutput=True)
    weighted = nc.declare_dram_parameter("weighted", [NQL, D], F32, isOutput=True)

    with tile.TileContext(nc) as tc:
        from contextlib import ExitStack

        with ExitStack() as octx:
            cpool = octx.enter_context(tc.tile_pool(name="const", bufs=1))
            ident = cpool.tile([P, P], F32)
            masks.make_identity(nc, ident[:])

            atp = octx.enter_context(tc.tile_pool(name="attnT", bufs=1))
            attnT = atp.tile([P, NKT, NQL], F32R)  # [k-part, k-tile, q]

            stats = octx.enter_context(tc.tile_pool(name="stats", bufs=1))
            mp = stats.tile([P, NQT, NPAN], F32)      # per-panel row maxes
            negmp = stats.tile([P, NQT, NPAN], F32)   # -mp (exp bias)
            psums = stats.tile([P, NQT, NPAN], F32)   # per-panel exp row sums
            mfin = stats.tile([P, NQT], F32)
            dlt = stats.tile([P, NQT, NPAN], F32)     # mp - mfin
            epan = stats.tile([P, NQT, NPAN], F32)    # exp(mp - mfin)
            wpan = stats.tile([P, NQT, NPAN], F32)    # epan * psums
            ssum = stats.tile([P, NQT], F32)
            recip = stats.tile([P, NQT], F32)
            scl = stats.tile([P, NQT, NPAN], F32)     # epan / ssum

            kldp = octx.enter_context(tc.tile_pool(name="kld", bufs=1))
            vrp = octx.enter_context(tc.tile_pool(name="vr", bufs=3))

            psx = ExitStack()
            scp = psx.enter_context(tc.tile_pool(name="scores", bufs=1))
            psmm = psx.enter_context(tc.tile_pool(name="psmm", bufs=2, space="PSUM"))
            pstp = psx.enter_context(tc.tile_pool(name="pstp", bufs=4, space="PSUM"))

            with ExitStack() as actx:
                qtp = actx.enter_context(tc.tile_pool(name="qt", bufs=1))
                ktp = actx.enter_context(tc.tile_pool(name="kt", bufs=2))

                # ---- phase 0: load Q^T (pre-transposed on host), round ----
                QT = qtp.tile([P, NDC, NQL], F32R)   # [d-part, d-chunk, q]

                qT_r = qT.rearrange("(c p) q -> p c q", p=P)
                kT_r = kT.rearrange("(c p) k -> p c k", p=P)

                def load_qt_pair(dcp):
                    # two d-chunks per DMA (4KB/partition)
                    qst = kldp.tile([P, 2, NQL], F32, tag=f"kld{(dcp // 2) % 4}", name="qst")
                    nc.sync.dma_start(qst[:], qT_r[:, dcp:dcp + 2, :])
                    if dcp % 4 == 0:
                        nc.vector.tensor_copy(QT[:, dcp:dcp + 2, :], qst[:])
                    else:
                        nc.scalar.copy(QT[:, dcp:dcp + 2, :], qst[:])

                # ---- phase A: per-panel K^T load+round, then mm1 ----
                scores = [
                    scp.tile([P, NK], F32, tag=f"sc{qi}", name=f"sc{qi}") for qi in range(NQT)
                ]

                def load_round_pair(kt_tile, pa, dcp):
                    # two d-chunks per DMA (2x2KB strided per partition)
                    kst = kldp.tile([P, 2, PANEL], F32, tag=f"kld{(dcp // 2) % 4}", name="kst")
                    nc.sync.dma_start(
                        kst[:],
                        kT_r[:, dcp:dcp + 2, pa * PANEL:(pa + 1) * PANEL],
                    )
                    if dcp % 4 == 0:
                        nc.vector.tensor_copy(kt_tile[:, dcp:dcp + 2, :], kst[:])
                    else:
                        nc.scalar.copy(kt_tile[:, dcp:dcp + 2, :], kst[:])

                def load_round_panel(kt_tile, pa):
                    for dcp in range(0, NDC, 2):
                        load_round_pair(kt_tile, pa, dcp)

                def mm1_quarter(kt_tile, pa, qi, half):
                    # 4 accumulating matmuls (half a d-sweep) for one q tile
                    s = mm_psum[qi]
                    for dc in range(half * 4, half * 4 + 4):
                        nc.tensor.matmul(
                            s[:],
                            QT[:, dc, qi * P:(qi + 1) * P],
                            kt_tile[:, dc, :],
                            start=(dc == 0),
                            stop=(dc == NDC - 1),
                        )
                    if half == 1:
                        nc.vector.reduce_max(
                            mp[:, qi, pa:pa + 1], s[:], axis=mybir.AxisListType.X
                        )
                        nc.vector.tensor_scalar_mul(
                            negmp[:, qi, pa:pa + 1], mp[:, qi, pa:pa + 1], -1.0
                        )
                        nc.scalar.activation(
                            scores[qi][:, pa * PANEL:(pa + 1) * PANEL],
                            s[:],
                            mybir.ActivationFunctionType.Exp,
                            bias=negmp[:, qi, pa:pa + 1],
                            scale=1.0,
                            accum_out=psums[:, qi, pa:pa + 1],
                        )

                mm_psum = {}
                KT_tiles = {}
                KT_tiles[0] = ktp.tile([P, NDC, PANEL], F32R, tag="kt", name="kt0")
                # first-needed chunks first: K panel-0 dc 0-3 and Q dc 0-3,
                # so the first mm1 quarter can start after ~4 fat transfers
                for dcp in (0, 2):
                    load_round_pair(KT_tiles[0], 0, dcp)
                for dcp in (0, 2):
                    load_qt_pair(dcp)
                for dcp in (4, 6):
                    load_round_pair(KT_tiles[0], 0, dcp)
                for dcp in (4, 6):
                    load_qt_pair(dcp)
                for pa in range(NPAN):
                    if pa + 1 < NPAN:
                        KT_tiles[pa + 1] = ktp.tile(
                            [P, NDC, PANEL], F32R, tag="kt", name=f"kt{pa+1}"
                        )
                        load_round_panel(KT_tiles[pa + 1], pa + 1)
                    # fresh psum accumulators per panel
                    for qi in range(NQT):
                        mm_psum[qi] = psmm.tile(
                            [P, PANEL], F32, tag=f"mm1_{qi % 2}", name=f"s{qi}"
                        )
                    for g in range(8):
                        qi, half = divmod(g, 2)
                        mm1_quarter(KT_tiles[pa], pa, qi, half)
                    del KT_tiles[pa]

                # ---- softmax + attn transpose, per q tile (overlapped) ----
                for qi in range(NQT):
                    nc.vector.reduce_max(
                        mfin[:, qi:qi + 1], mp[:, qi, :], axis=mybir.AxisListType.X
                    )
                    # scale_p = exp(mp - mfin) * psums normalization factors
                    nc.vector.tensor_scalar(
                        dlt[:, qi, :], mp[:, qi, :], mfin[:, qi:qi + 1], None,
                        op0=mybir.AluOpType.subtract,
                    )
                    nc.scalar.activation(
                        epan[:, qi, :], dlt[:, qi, :],
                        mybir.ActivationFunctionType.Exp,
                    )
                    nc.vector.tensor_tensor(
                        wpan[:, qi, :], epan[:, qi, :], psums[:, qi, :],
                        op=mybir.AluOpType.mult,
                    )
                    nc.vector.reduce_sum(
                        ssum[:, qi:qi + 1], wpan[:, qi, :], axis=mybir.AxisListType.X
                    )
                    nc.vector.reciprocal(recip[:, qi:qi + 1], ssum[:, qi:qi + 1])
                    nc.vector.tensor_scalar_mul(
                        scl[:, qi, :], epan[:, qi, :], recip[:, qi:qi + 1]
                    )
                    # rescale + transpose per 512-col chunk (chunk == panel)
                    for group in range(NKT // 4):
                        sl = slice(group * 4 * P, (group + 1) * 4 * P)
                        eng = (
                            nc.vector.tensor_scalar_mul
                            if group % 2 == 0
                            else nc.scalar.mul
                        )
                        eng(
                            scores[qi][:, sl], scores[qi][:, sl],
                            scl[:, qi, group:group + 1],
                        )
                        pt = pstp.tile([P, 4 * P], F32, tag="tp", name="pt")
                        for j in range(4):
                            kt_i = group * 4 + j
                            nc.tensor.transpose(
                                pt[:, j * P:(j + 1) * P],
                                scores[qi][:, kt_i * P:(kt_i + 1) * P],
                                ident[:],
                            )
                        eng = nc.vector.tensor_copy if group % 2 == 0 else nc.scalar.copy
                        eng(
                            attnT[:, group * 4:(group + 1) * 4, qi * P:(qi + 1) * P],
                            pt[:].rearrange("p (j q) -> p j q", j=4),
                        )
                        if qi == NQT - 1:
                            # tile rows complete once the last q block lands
                            for j in range(4):
                                kt_i = group * 4 + j
                                nc.gpsimd.dma_start(
                                    attn[kt_i * P:(kt_i + 1) * P, :],
                                    attnT[:, kt_i, :].bitcast(F32),
                                )
            psx.close()

            # ---- phase B: weighted = attnT.T @ V ----
            with ExitStack() as bctx:
                woutp = bctx.enter_context(tc.tile_pool(name="wout", bufs=4))
                psb = bctx.enter_context(
                    tc.tile_pool(name="psb", bufs=1, space="PSUM")
                )
                acc = [
                    [psb.tile([P, EH], F32, tag=f"acc{eh}_{qi}", name=f"acc{eh}_{qi}") for qi in range(NQT)]
                    for eh in range(NEH)
                ]
                for kt_i in range(NKT):
                    vld = kldp.tile([P, D], F32, tag=f"kld{kt_i % 4}", name="vld")
                    nc.sync.dma_start(vld[:], values[kt_i * P:(kt_i + 1) * P, :])
                    vr = vrp.tile([P, D], F32R, tag="vr")
                    if kt_i % 2 == 0:
                        nc.vector.tensor_copy(vr[:], vld[:])
                    else:
                        nc.scalar.copy(vr[:], vld[:])
                    for eh in range(NEH):
                        for qi in range(NQT):
                            nc.tensor.matmul(
                                acc[eh][qi][:],
                                attnT[:, kt_i, qi * P:(qi + 1) * P],
                                vr[:, eh * EH:(eh + 1) * EH],
                                start=(kt_i == 0),
                                stop=(kt_i == NKT - 1),
                            )
                for eh in range(NEH):
                    for qi in range(NQT):
                        wout = woutp.tile([P, EH], F32, tag="wout")
                        nc.vector.tensor_copy(wout[:], acc[eh][qi][:])
                        nc.scalar.dma_start(
                            weighted[qi * P:(qi + 1) * P, eh * EH:(eh + 1) * EH],
                            wout[:],
                        )
    return nc


_NC_CACHE = {}


def _get_nc():
    if "nc" not in _NC_CACHE:
        _NC_CACHE["nc"] = build_attention_nc()
    return _NC_CACHE["nc"]


def make_in_maps(queries, keys, values):
    keys_T = np.ascontiguousarray(keys.T)
    queries_T = np.ascontiguousarray(queries.T)
    return [
        {
            "qT": np.ascontiguousarray(queries_T[:, c * NQL:(c + 1) * NQL]),
            "kT": keys_T,
            "values": values,
        }
        for c in range(N_CORES)
    ]


def kernel(queries, keys, values):
    queries = np.ascontiguousarray(np.asarray(queries, dtype=np.float32))
    keys = np.ascontiguousarray(np.asarray(keys, dtype=np.float32))
    values = np.ascontiguousarray(np.asarray(values, dtype=np.float32))
    assert queries.shape == (NQ, D) and keys.shape == (NK, D)
    assert values.shape == (NK, D)

    nc = _get_nc()
    in_maps = make_in_maps(queries, keys, values)
    res = run_bass_kernel_spmd(nc, in_maps, list(range(N_CORES)))
    attn_full = np.concatenate(
        [res.results[c]["attn"] for c in range(N_CORES)], axis=1
    )
    weighted_full = np.concatenate(
        [res.results[c]["weighted"] for c in range(N_CORES)], axis=0
    )
    return (weighted_full, attn_full)
